# revision 30
# baseline (speedup 1.0000x reference)
"""RAFT-style CorrBlock kernel for Trainium2 (8 NeuronCores, Bass/Tile).

Full inputs: fmap1 [2,256,64,64], fmap2 [2,256,64,64], centroids_coords [2,2,64,64].
Output: [2, 324, 64, 64] f32.

Sharding: data-parallel over the B*H1*W1 query-pixel axis. Core c handles batch
c//4, query pixels (c%4)*1024 .. +1024.

v3: bf16 matmul/slab/combine pipeline, two DRAM slab tensors per pixel-group
(levels 1-3 written first so their band gathers stream while the level-0 chunks
are still on the PE), inputs loaded small-first so the first matmul starts ~4us
in, indirect-gather indices pre-offset by the guard size (HW DGE drops negative
raw indices). Host pre-pools f2 (sums) into one [256, 5440] bf16 operand and
casts the bf16 output back to f32.
"""

import numpy as np
import ml_dtypes

import concourse.bass as bass
import concourse.bacc as bacc
import concourse.mybir as mybir
import concourse.tile as tile
from concourse.bass_utils import run_bass_kernel_spmd

f32 = mybir.dt.float32
bf16 = mybir.dt.bfloat16
i32 = mybir.dt.int32
OP = mybir.AluOpType

P = 128
C = 256
HW = 4096
NPIX = 1024
NG = NPIX // P     # 8 groups of 128 pixels
NLVL = 4
S = 9              # sample window side (2*RADIUS+1)
PS = 10            # patch side
W_L = [64, 32, 16, 8]
HW_L = [w * w for w in W_L]           # 4096, 1024, 256, 64
B_L = [9 * w + PS for w in W_L]       # band length: 586, 298, 154, 82
FEAT = NLVL * S * S                   # 324

# Slab rows carry their own zero pads (written with the data by the same DMA
# the gathers depend on -- no separate guard-fill DMA to race against).
PADA0, PADA1 = 260, 328               # max under/overflow of a level-0 band
ROWA = PADA0 + HW_L[0] + PADA1        # 4684 per-pixel level-0 row
PADB0, PADB1 = 132, 48                # max under/overflow of level 1-3 bands
ROWB = PADB0 + HW_L[1] + HW_L[2] + HW_L[3] + PADB1   # 1524
RBB = [PADB0, PADB0 + HW_L[1], PADB0 + HW_L[1] + HW_L[2]]
NTA = P * ROWA
NTB = P * ROWB
F2COLS = sum(HW_L)                    # 5440 in the concatenated f2 operand


def _ap_view(t_ap, offset, dims):
    """Arbitrary strided view of a tile AP: dims = [[step, count], ...] free dims."""
    return bass.AP(t_ap.tensor, t_ap.offset + offset, [list(t_ap.ap[0])] + dims)


def build_bass():
    nc = bacc.Bacc("TRN2", target_bir_lowering=False, debug=False)

    f1_d = nc.dram_tensor("f1", [C, NPIX], bf16, kind="ExternalInput")
    f2_d = nc.dram_tensor("f2a", [C, F2COLS], bf16, kind="ExternalInput")
    ccx_d = nc.dram_tensor("ccx", [P, NG], f32, kind="ExternalInput")
    ccy_d = nc.dram_tensor("ccy", [P, NG], f32, kind="ExternalInput")
    out_d = nc.dram_tensor("out", [NPIX, FEAT], bf16, kind="ExternalOutput")
    slabA = [nc.dram_tensor(f"slabA{g}", [NTA], bf16) for g in range(NG)]
    slabB = [nc.dram_tensor(f"slabB{g}", [NTB], bf16) for g in range(NG)]

    with tile.TileContext(nc) as tc:
        with (
            tc.tile_pool(name="persist", bufs=1) as pp,
            tc.tile_pool(name="grp", bufs=2) as pg,
            tc.tile_pool(name="psA", bufs=3, space="PSUM") as psA,
            tc.tile_pool(name="psB", bufs=2, space="PSUM") as psB,
        ):
            # ---- input loads, small and L123-f2 first so compute starts early ----
            ccx = pp.tile([P, NG], f32, tag="ccx")
            ccy = pp.tile([P, NG], f32, tag="ccy")
            nc.sync.dma_start(ccx[:], ccx_d.ap())
            nc.sync.dma_start(ccy[:], ccy_d.ap())
            f1t = []
            for k in range(2):
                t1 = pp.tile([P, NPIX], bf16, tag=f"f1_{k}", name=f"f1_{k}")
                nc.sync.dma_start(t1[:], f1_d.ap()[k * P:(k + 1) * P, :])
                f1t.append(t1)
            f2B = []
            for k in range(2):
                tb = pp.tile([P, 1344], bf16, tag=f"f2B_{k}", name=f"f2B_{k}")
                nc.sync.dma_start(tb[:], f2_d.ap()[k * P:(k + 1) * P, HW_L[0]:F2COLS])
                f2B.append(tb)
            f2A = []
            for k in range(2):
                ta = pp.tile([P, HW_L[0]], bf16, tag=f"f2A_{k}", name=f"f2A_{k}")
                nc.scalar.dma_start(ta[:], f2_d.ap()[k * P:(k + 1) * P, 0:HW_L[0]])
                f2A.append(ta)

            # ---- persistent corr buffers with zeroed pad columns ----
            corrAb = []
            corrBb = []
            for i in range(2):
                ca = pp.tile([P, ROWA], bf16, tag=f"corrA_{i}", name=f"corrA_{i}")
                nc.vector.memset(ca[:, 0:PADA0], 0.0)
                nc.vector.memset(ca[:, ROWA - PADA1:ROWA], 0.0)
                corrAb.append(ca)
                cb = pp.tile([P, ROWB], bf16, tag=f"corrB_{i}", name=f"corrB_{i}")
                nc.vector.memset(cb[:, 0:PADB0], 0.0)
                nc.vector.memset(cb[:, ROWB - PADB1:ROWB], 0.0)
                corrBb.append(cb)

            # ---- per-level index / weight / mask precompute (all f32) ----
            kvi = pp.tile([P, NG * PS], i32, tag="kvi")
            nc.gpsimd.iota(kvi[:], pattern=[[0, NG], [1, PS]], base=0, channel_multiplier=0)
            kvf = pp.tile([P, NG * PS], f32, tag="kvf")
            nc.vector.tensor_copy(out=kvf[:], in_=kvi[:])
            pf_i = pp.tile([P, NG], i32, tag="pf_i")
            nc.gpsimd.iota(pf_i[:], pattern=[[0, NG]], base=0, channel_multiplier=1)
            pf_f = pp.tile([P, NG], f32, tag="pf_f")
            nc.vector.tensor_copy(out=pf_f[:], in_=pf_i[:])

            idx_l = []
            w4 = [pp.tile([P, NG * NLVL], f32, tag=f"w4_{t}", name=f"w4_{t}")
                  for t in range(4)]
            m_l = []     # [P, NG*100] bf16 patch validity masks
            for l in range(NLVL):
                wl = W_L[l]
                inv = 1.0 / (1 << l)
                sc = 1.0 / (16.0 * (4.0 ** l))
                rowl = ROWA if l == 0 else ROWB
                basel = (PADA0 if l == 0 else RBB[l - 1]) - 4 * wl - 4

                xs = pg.tile([P, NG], f32, tag="xs")
                ys = pg.tile([P, NG], f32, tag="ys")
                nc.vector.tensor_scalar_mul(xs[:], ccx[:], inv)
                nc.vector.tensor_scalar_mul(ys[:], ccy[:], inv)

                def floor_of(v, nm):
                    ti = pg.tile([P, NG], i32, tag=f"fl_i_{nm}")
                    nc.vector.tensor_copy(out=ti[:], in_=v[:])
                    tf = pg.tile([P, NG], f32, tag=f"fl_f_{nm}")
                    nc.vector.tensor_copy(out=tf[:], in_=ti[:])
                    gt = pg.tile([P, NG], f32, tag=f"fl_g_{nm}")
                    nc.vector.tensor_tensor(out=gt[:], in0=tf[:], in1=v[:], op=OP.is_gt)
                    fl = pg.tile([P, NG], f32, tag=f"fl_o_{nm}")
                    nc.vector.tensor_tensor(out=fl[:], in0=tf[:], in1=gt[:], op=OP.subtract)
                    return fl

                x0 = floor_of(xs, "x")
                y0 = floor_of(ys, "y")

                fx = pg.tile([P, NG], f32, tag="fx")
                fy = pg.tile([P, NG], f32, tag="fy")
                nc.vector.tensor_tensor(out=fx[:], in0=xs[:], in1=x0[:], op=OP.subtract)
                nc.vector.tensor_tensor(out=fy[:], in0=ys[:], in1=y0[:], op=OP.subtract)

                # weights: w_ab = wy_a * wx_b * sc ; wx1 = fx, wx0 = 1-fx
                wy0s = pg.tile([P, NG], f32, tag="wy0s")
                wy1s = pg.tile([P, NG], f32, tag="wy1s")
                nc.vector.tensor_scalar(wy0s[:], fy[:], -sc, sc, OP.mult, OP.add)
                nc.vector.tensor_scalar_mul(wy1s[:], fy[:], sc)
                wx0 = pg.tile([P, NG], f32, tag="wx0")
                nc.vector.tensor_scalar(wx0[:], fx[:], -1.0, 1.0, OP.mult, OP.add)
                for t, (wya, wxb) in enumerate(
                    ((wy0s, wx0), (wy0s, fx), (wy1s, wx0), (wy1s, fx))
                ):
                    # layout [P, (g l)]: stride NLVL per group, offset l
                    nc.vector.tensor_tensor(
                        out=_ap_view(w4[t][:], l, [[NLVL, NG]]),
                        in0=wya[:], in1=wxb[:], op=OP.mult,
                    )

                # band start: p*row + pad + RB + (y0-4)*W_l + (x0-4)  (always >= 0:
                # HW DGE silently drops descriptors with negative raw indices)
                t1 = pg.tile([P, NG], f32, tag="idx_t1")
                nc.vector.scalar_tensor_tensor(
                    out=t1[:], in0=y0[:], scalar=float(wl), in1=x0[:],
                    op0=OP.mult, op1=OP.add,
                )
                t2 = pg.tile([P, NG], f32, tag="idx_t2")
                nc.vector.scalar_tensor_tensor(
                    out=t2[:], in0=pf_f[:], scalar=float(rowl), in1=t1[:],
                    op0=OP.mult, op1=OP.add,
                )
                t3 = pg.tile([P, NG], f32, tag="idx_t3")
                nc.vector.tensor_scalar_add(t3[:], t2[:], float(basel))
                ii = pp.tile([P, NG], i32, tag=f"idx_{l}")
                nc.vector.tensor_copy(out=ii[:], in_=t3[:])
                idx_l.append(ii)

                # row/col validity: valid iff 4-k <= c0 <= wl+3-k
                def valid(c0, lim, nm):
                    tt = pg.tile([P, NG * PS], f32, tag=f"v_t_{nm}")
                    nc.vector.tensor_tensor(
                        out=tt[:].rearrange("p (g k) -> p g k", k=PS),
                        in0=kvf[:].rearrange("p (g k) -> p g k", k=PS),
                        in1=c0[:, :, None].to_broadcast([P, NG, PS]),
                        op=OP.add,
                    )
                    c1 = pg.tile([P, NG * PS], f32, tag=f"v_c_{nm}")
                    nc.vector.tensor_scalar(c1[:], tt[:], 4.0, None, OP.is_ge)
                    vv = pg.tile([P, NG * PS], f32, tag=f"v_o_{nm}")
                    nc.vector.scalar_tensor_tensor(
                        out=vv[:], in0=tt[:], scalar=float(lim + 3), in1=c1[:],
                        op0=OP.is_le, op1=OP.mult,
                    )
                    return vv

                rv = valid(y0, wl, "r")
                cv = valid(x0, wl, "c")
                mm = pp.tile([P, NG * PS * PS], bf16, tag=f"m_{l}")
                nc.vector.tensor_tensor(
                    out=mm[:].rearrange("p (g a b) -> p g a b", a=PS, b=PS),
                    in0=rv[:].rearrange("p (g k) -> p g k", k=PS)[:, :, :, None]
                        .to_broadcast([P, NG, PS, PS]),
                    in1=cv[:].rearrange("p (g k) -> p g k", k=PS)[:, :, None, :]
                        .to_broadcast([P, NG, PS, PS]),
                    op=OP.mult,
                )
                m_l.append(mm)

            # ---- main pipeline over the 8 groups ----
            # k-swept matmul blocks: within a block, all chunks run k=0 then all
            # k=1 so the stationary f1 tile is loaded twice per block instead of
            # per-matmul (LDWEIGHTS sits on the PE pipeline between matmuls).
            def mm_block(g, chunks):
                # chunks: list of (psum_ap, rhs_tile, rhs_off, width)
                for k in range(2):
                    for pt, rhs, c0, wid in chunks:
                        for s0 in range(0, wid, 512):
                            sw = min(512, wid - s0)
                            nc.tensor.matmul(
                                out=pt[:, s0:s0 + sw],
                                lhsT=f1t[k][:, g * P:(g + 1) * P],
                                rhs=rhs[k][:, c0 + s0:c0 + s0 + sw],
                                start=(k == 0), stop=(k == 1),
                            )

            def do_B(g, corrB):
                ptB1 = psA.tile([P, 1024], f32, tag="mmA", name=f"mmB1_{g}")
                ptB2 = psB.tile([P, 320], f32, tag="mmB", name=f"mmB2_{g}")
                mm_block(g, [(ptB1, f2B, 0, 1024), (ptB2, f2B, 1024, 320)])
                nc.scalar.copy(out=corrB[:, PADB0:PADB0 + 1024], in_=ptB1[:])
                nc.vector.tensor_copy(
                    out=corrB[:, PADB0 + 1024:PADB0 + 1344], in_=ptB2[:])
                nc.sync.dma_start(
                    slabB[g].ap().rearrange("(p f) -> p f", f=ROWB), corrB[:])

            def do_A(g, corrA):
                # two blocks of two 1024-chunks (psA bufs=3 caps live tiles)
                for blk in range(2):
                    pts = [psA.tile([P, 1024], f32, tag="mmA", name=f"mmA_{g}_{blk}_{ci}")
                           for ci in range(2)]
                    mm_block(g, [(pts[ci], f2A, (blk * 2 + ci) * 1024, 1024)
                                 for ci in range(2)])
                    for ci in range(2):
                        c0 = PADA0 + (blk * 2 + ci) * 1024
                        if blk * 2 + ci < 3:
                            nc.scalar.copy(out=corrA[:, c0:c0 + 1024], in_=pts[ci][:])
                        else:
                            nc.vector.tensor_copy(out=corrA[:, c0:c0 + 1024], in_=pts[ci][:])
                nc.sync.dma_start(
                    slabA[g].ap().rearrange("(p f) -> p f", f=ROWA), corrA[:])

            for g in range(NG):
                corrB = corrBb[g % 2]
                corrA = corrAb[g % 2]
                if g < NG - 1:
                    # levels 1-3 first: small matmuls, slabB write, gathers
                    # stream while the level-0 chunks run on the PE
                    do_B(g, corrB)
                    do_A(g, corrA)
                else:
                    # last group: level 0 first so the tail chain is the short
                    # slabB write -> small gathers -> combine
                    do_A(g, corrA)
                    do_B(g, corrB)

                # band gathers (levels 1-3 from slabB first, then level 0; the
                # last group gathers level 0 first to shorten the tail) + pm
                pm4 = pg.tile([P, NLVL * PS * PS], bf16, tag="pm4", name=f"pm4_{g}")
                feats = pg.tile([P, FEAT], bf16, tag="feats", name=f"feats_{g}")

                def gather_pm(l):
                    bl = B_L[l]
                    wl = W_L[l]
                    src = slabA[g] if l == 0 else slabB[g]
                    band = pg.tile([P, bl], bf16, tag=f"band_{l}", name=f"band_{l}_{g}")
                    nc.gpsimd.indirect_dma_start(
                        out=band[:],
                        out_offset=None,
                        in_=src.ap()[:, None],
                        in_offset=bass.IndirectOffsetOnAxis(ap=idx_l[l][:, g:g + 1], axis=0),
                        element_offset=0,
                    )
                    nc.vector.tensor_tensor(
                        out=_ap_view(pm4[:], l * PS * PS, [[PS, PS], [1, PS]]),
                        in0=_ap_view(band[:], 0, [[wl, PS], [1, PS]]),
                        in1=_ap_view(m_l[l][:], g * PS * PS, [[PS, PS], [1, PS]]),
                        op=OP.mult,
                    )

                def taps(l0, nl):
                    # 4-tap bilinear combine over levels [l0, l0+nl), all bf16
                    ov = _ap_view(feats[:], l0 * S * S, [[S * S, nl], [S, S], [1, S]])
                    for t, (a, b) in enumerate(((0, 0), (0, 1), (1, 0), (1, 1))):
                        # feature index = i*9 + j with i = x-offset (outer), j =
                        # y-offset (inner); patch [y=j+a, x=i+b] at (j+a)*10+(i+b)
                        pv = _ap_view(pm4[:], l0 * PS * PS + a * PS + b,
                                      [[PS * PS, nl], [1, S], [PS, S]])
                        wb = _ap_view(w4[t][:], g * NLVL + l0,
                                      [[1, nl], [0, S], [0, S]])
                        if t == 0:
                            nc.vector.tensor_tensor(out=ov, in0=pv, in1=wb, op=OP.mult)
                        else:
                            tmp = pg.tile([P, FEAT], bf16, tag="cmb_tmp")
                            tv = _ap_view(tmp[:], l0 * S * S, [[S * S, nl], [S, S], [1, S]])
                            nc.vector.tensor_tensor(out=tv, in0=pv, in1=wb, op=OP.mult)
                            nc.vector.tensor_tensor(out=ov, in0=ov, in1=tv, op=OP.add)

                if g < NG - 1:
                    for l in (1, 2, 3, 0):
                        gather_pm(l)
                    taps(0, NLVL)
                else:
                    # last group: level-0 combine overlaps the B matmuls, and the
                    # tail chain is only the short slabB write -> B gathers -> taps
                    gather_pm(0)
                    taps(0, 1)
                    for l in (1, 2, 3):
                        gather_pm(l)
                    taps(1, NLVL - 1)

                # out-write trigger on gpsimd: its wait-for-taps lands after the
                # gathers it trails anyway; on scalar/sync it would head-of-line
                # block the next group's PSUM copies or slab writes
                nc.gpsimd.dma_start(out_d.ap()[g * P:(g + 1) * P, :], feats[:])

    nc.compile()
    return nc


_NC = None


def _get_nc():
    global _NC
    if _NC is None:
        _NC = build_bass()
    return _NC


def _pool_f2(f2b):
    """f2b: [C, 64, 64] f32 -> [C, 5440] level-concatenated pooled SUMS."""
    lvls = [f2b.reshape(C, HW)]
    cur = f2b
    for _ in range(1, NLVL):
        c, h, w = cur.shape
        cur = cur.reshape(c, h // 2, 2, w // 2, 2).sum(axis=(2, 4))
        lvls.append(cur.reshape(c, -1))
    return np.concatenate(lvls, axis=1)


def make_in_maps(fmap1, fmap2, centroids_coords):
    bf = ml_dtypes.bfloat16
    f2a = [np.ascontiguousarray(_pool_f2(np.asarray(fmap2[bi], dtype=np.float32))).astype(bf)
           for bi in range(2)]
    in_maps = []
    for core in range(8):
        bi, chunk = divmod(core, 4)
        m0 = chunk * NPIX
        f1 = np.ascontiguousarray(
            fmap1[bi].reshape(C, HW)[:, m0:m0 + NPIX]).astype(bf)
        cc = centroids_coords[bi].reshape(2, HW)[:, m0:m0 + NPIX]
        ccx = np.ascontiguousarray(cc[0].reshape(NG, P).T, dtype=np.float32)  # [p, g]
        ccy = np.ascontiguousarray(cc[1].reshape(NG, P).T, dtype=np.float32)
        in_maps.append({"f1": f1, "f2a": f2a[bi], "ccx": ccx, "ccy": ccy})
    return in_maps


def assemble(outs):
    """outs: list of 8 arrays [1024, 324] bf16 -> [2, 324, 64, 64] f32"""
    full = np.empty((2, FEAT, 64, 64), dtype=np.float32)
    for bi in range(2):
        feats = np.concatenate(
            [np.asarray(outs[bi * 4 + c], dtype=np.float32) for c in range(4)], axis=0)
        full[bi] = feats.reshape(64, 64, FEAT).transpose(2, 0, 1)
    return full


def kernel(fmap1, fmap2, centroids_coords, trace=False):
    nc = _get_nc()
    in_maps = make_in_maps(fmap1, fmap2, centroids_coords)
    try:
        res = run_bass_kernel_spmd(nc, in_maps, core_ids=list(range(8)), trace=trace)
    except ModuleNotFoundError:
        res = run_bass_kernel_spmd(nc, in_maps, core_ids=list(range(8)), trace=False)
    out = assemble([r["out"] for r in res.results])
    if trace:
        kernel.last_result = res
    return out


# revision 31
# speedup vs baseline: 1.0393x; 1.0393x over previous
"""RAFT-style CorrBlock kernel for Trainium2 (8 NeuronCores, Bass/Tile).

Full inputs: fmap1 [2,256,64,64], fmap2 [2,256,64,64], centroids_coords [2,2,64,64].
Output: [2, 324, 64, 64] f32.

Sharding: data-parallel over the B*H1*W1 query-pixel axis. Core c handles batch
c//4, query pixels (c%4)*1024 .. +1024.

v3: bf16 matmul/slab/combine pipeline, two DRAM slab tensors per pixel-group
(levels 1-3 written first so their band gathers stream while the level-0 chunks
are still on the PE), inputs loaded small-first so the first matmul starts ~4us
in, indirect-gather indices pre-offset by the guard size (HW DGE drops negative
raw indices). Host pre-pools f2 (sums) into one [256, 5440] bf16 operand and
casts the bf16 output back to f32.
"""

import numpy as np
import ml_dtypes

import concourse.bass as bass
import concourse.bacc as bacc
import concourse.mybir as mybir
import concourse.tile as tile
from concourse.bass_utils import run_bass_kernel_spmd

f32 = mybir.dt.float32
bf16 = mybir.dt.bfloat16
i32 = mybir.dt.int32
OP = mybir.AluOpType

P = 128
C = 256
HW = 4096
NPIX = 1024
NG = NPIX // P     # 8 groups of 128 pixels
NLVL = 4
S = 9              # sample window side (2*RADIUS+1)
PS = 10            # patch side
W_L = [64, 32, 16, 8]
HW_L = [w * w for w in W_L]           # 4096, 1024, 256, 64
B_L = [9 * w + PS for w in W_L]       # band length: 586, 298, 154, 82
FEAT = NLVL * S * S                   # 324

# Slab rows carry their own zero pads (written with the data by the same DMA
# the gathers depend on -- no separate guard-fill DMA to race against).
PADA0, PADA1 = 260, 328               # max under/overflow of a level-0 band
ROWA = PADA0 + HW_L[0] + PADA1        # 4684 per-pixel level-0 row
PADB0, PADB1 = 132, 48                # max under/overflow of level 1-3 bands
ROWB = PADB0 + HW_L[1] + HW_L[2] + HW_L[3] + PADB1   # 1524
RBB = [PADB0, PADB0 + HW_L[1], PADB0 + HW_L[1] + HW_L[2]]
NTA = P * ROWA
NTB = P * ROWB
F2COLS = sum(HW_L)                    # 5440 in the concatenated f2 operand


def _ap_view(t_ap, offset, dims):
    """Arbitrary strided view of a tile AP: dims = [[step, count], ...] free dims."""
    return bass.AP(t_ap.tensor, t_ap.offset + offset, [list(t_ap.ap[0])] + dims)


def build_bass():
    nc = bacc.Bacc("TRN2", target_bir_lowering=False, debug=False)

    f1_d = nc.dram_tensor("f1", [C, NPIX], bf16, kind="ExternalInput")
    f2_d = nc.dram_tensor("f2a", [C, F2COLS], bf16, kind="ExternalInput")
    ccx_d = nc.dram_tensor("ccx", [P, NG], f32, kind="ExternalInput")
    ccy_d = nc.dram_tensor("ccy", [P, NG], f32, kind="ExternalInput")
    out_d = nc.dram_tensor("out", [NPIX, FEAT], bf16, kind="ExternalOutput")
    slabA = [nc.dram_tensor(f"slabA{g}", [NTA], bf16) for g in range(NG)]
    slabB = [nc.dram_tensor(f"slabB{g}", [NTB], bf16) for g in range(NG)]

    with tile.TileContext(nc) as tc:
        with (
            tc.tile_pool(name="persist", bufs=1) as pp,
            tc.tile_pool(name="grp", bufs=2) as pg,
            tc.tile_pool(name="psA", bufs=3, space="PSUM") as psA,
            tc.tile_pool(name="psB", bufs=2, space="PSUM") as psB,
        ):
            # ---- input loads, small and L123-f2 first so compute starts early ----
            ccx = pp.tile([P, NG], f32, tag="ccx")
            ccy = pp.tile([P, NG], f32, tag="ccy")
            nc.sync.dma_start(ccx[:], ccx_d.ap())
            nc.sync.dma_start(ccy[:], ccy_d.ap())
            f1t = []
            for k in range(2):
                t1 = pp.tile([P, NPIX], bf16, tag=f"f1_{k}", name=f"f1_{k}")
                nc.sync.dma_start(t1[:], f1_d.ap()[k * P:(k + 1) * P, :])
                f1t.append(t1)
            f2B = []
            for k in range(2):
                tb = pp.tile([P, 1344], bf16, tag=f"f2B_{k}", name=f"f2B_{k}")
                nc.sync.dma_start(tb[:], f2_d.ap()[k * P:(k + 1) * P, HW_L[0]:F2COLS])
                f2B.append(tb)
            f2A = []
            for k in range(2):
                ta = pp.tile([P, HW_L[0]], bf16, tag=f"f2A_{k}", name=f"f2A_{k}")
                nc.scalar.dma_start(ta[:], f2_d.ap()[k * P:(k + 1) * P, 0:HW_L[0]])
                f2A.append(ta)

            # ---- persistent corr buffers with zeroed pad columns ----
            corrAb = []
            corrBb = []
            for i in range(2):
                ca = pp.tile([P, ROWA], bf16, tag=f"corrA_{i}", name=f"corrA_{i}")
                nc.vector.memset(ca[:, 0:PADA0], 0.0)
                nc.vector.memset(ca[:, ROWA - PADA1:ROWA], 0.0)
                corrAb.append(ca)
                cb = pp.tile([P, ROWB], bf16, tag=f"corrB_{i}", name=f"corrB_{i}")
                nc.vector.memset(cb[:, 0:PADB0], 0.0)
                nc.vector.memset(cb[:, ROWB - PADB1:ROWB], 0.0)
                corrBb.append(cb)

            # ---- per-level index / weight / mask precompute (all f32) ----
            kvi = pp.tile([P, NG * PS], i32, tag="kvi")
            nc.gpsimd.iota(kvi[:], pattern=[[0, NG], [1, PS]], base=0, channel_multiplier=0)
            kvf = pp.tile([P, NG * PS], f32, tag="kvf")
            nc.vector.tensor_copy(out=kvf[:], in_=kvi[:])
            pf_i = pp.tile([P, NG], i32, tag="pf_i")
            nc.gpsimd.iota(pf_i[:], pattern=[[0, NG]], base=0, channel_multiplier=1)
            pf_f = pp.tile([P, NG], f32, tag="pf_f")
            nc.vector.tensor_copy(out=pf_f[:], in_=pf_i[:])

            idx_l = []
            w4 = [pp.tile([P, NG * NLVL], f32, tag=f"w4_{t}", name=f"w4_{t}")
                  for t in range(4)]
            m_l = []     # [P, NG*100] bf16 patch validity masks
            for l in range(NLVL):
                wl = W_L[l]
                inv = 1.0 / (1 << l)
                sc = 1.0 / (16.0 * (4.0 ** l))
                rowl = ROWA if l == 0 else ROWB
                basel = (PADA0 if l == 0 else RBB[l - 1]) - 4 * wl - 4

                xs = pg.tile([P, NG], f32, tag="xs")
                ys = pg.tile([P, NG], f32, tag="ys")
                nc.vector.tensor_scalar_mul(xs[:], ccx[:], inv)
                nc.vector.tensor_scalar_mul(ys[:], ccy[:], inv)

                def floor_of(v, nm):
                    ti = pg.tile([P, NG], i32, tag=f"fl_i_{nm}")
                    nc.vector.tensor_copy(out=ti[:], in_=v[:])
                    tf = pg.tile([P, NG], f32, tag=f"fl_f_{nm}")
                    nc.vector.tensor_copy(out=tf[:], in_=ti[:])
                    gt = pg.tile([P, NG], f32, tag=f"fl_g_{nm}")
                    nc.vector.tensor_tensor(out=gt[:], in0=tf[:], in1=v[:], op=OP.is_gt)
                    fl = pg.tile([P, NG], f32, tag=f"fl_o_{nm}")
                    nc.vector.tensor_tensor(out=fl[:], in0=tf[:], in1=gt[:], op=OP.subtract)
                    return fl

                x0 = floor_of(xs, "x")
                y0 = floor_of(ys, "y")

                fx = pg.tile([P, NG], f32, tag="fx")
                fy = pg.tile([P, NG], f32, tag="fy")
                nc.vector.tensor_tensor(out=fx[:], in0=xs[:], in1=x0[:], op=OP.subtract)
                nc.vector.tensor_tensor(out=fy[:], in0=ys[:], in1=y0[:], op=OP.subtract)

                # weights: w_ab = wy_a * wx_b * sc ; wx1 = fx, wx0 = 1-fx
                wy0s = pg.tile([P, NG], f32, tag="wy0s")
                wy1s = pg.tile([P, NG], f32, tag="wy1s")
                nc.vector.tensor_scalar(wy0s[:], fy[:], -sc, sc, OP.mult, OP.add)
                nc.vector.tensor_scalar_mul(wy1s[:], fy[:], sc)
                wx0 = pg.tile([P, NG], f32, tag="wx0")
                nc.vector.tensor_scalar(wx0[:], fx[:], -1.0, 1.0, OP.mult, OP.add)
                for t, (wya, wxb) in enumerate(
                    ((wy0s, wx0), (wy0s, fx), (wy1s, wx0), (wy1s, fx))
                ):
                    # layout [P, (g l)]: stride NLVL per group, offset l
                    nc.vector.tensor_tensor(
                        out=_ap_view(w4[t][:], l, [[NLVL, NG]]),
                        in0=wya[:], in1=wxb[:], op=OP.mult,
                    )

                # band start: p*row + pad + RB + (y0-4)*W_l + (x0-4)  (always >= 0:
                # HW DGE silently drops descriptors with negative raw indices)
                t1 = pg.tile([P, NG], f32, tag="idx_t1")
                nc.vector.scalar_tensor_tensor(
                    out=t1[:], in0=y0[:], scalar=float(wl), in1=x0[:],
                    op0=OP.mult, op1=OP.add,
                )
                t2 = pg.tile([P, NG], f32, tag="idx_t2")
                nc.vector.scalar_tensor_tensor(
                    out=t2[:], in0=pf_f[:], scalar=float(rowl), in1=t1[:],
                    op0=OP.mult, op1=OP.add,
                )
                t3 = pg.tile([P, NG], f32, tag="idx_t3")
                nc.vector.tensor_scalar_add(t3[:], t2[:], float(basel))
                ii = pp.tile([P, NG], i32, tag=f"idx_{l}")
                nc.vector.tensor_copy(out=ii[:], in_=t3[:])
                idx_l.append(ii)

                # row/col validity: valid iff 4-k <= c0 <= wl+3-k
                def valid(c0, lim, nm):
                    tt = pg.tile([P, NG * PS], f32, tag=f"v_t_{nm}")
                    nc.vector.tensor_tensor(
                        out=tt[:].rearrange("p (g k) -> p g k", k=PS),
                        in0=kvf[:].rearrange("p (g k) -> p g k", k=PS),
                        in1=c0[:, :, None].to_broadcast([P, NG, PS]),
                        op=OP.add,
                    )
                    c1 = pg.tile([P, NG * PS], f32, tag=f"v_c_{nm}")
                    nc.vector.tensor_scalar(c1[:], tt[:], 4.0, None, OP.is_ge)
                    vv = pg.tile([P, NG * PS], f32, tag=f"v_o_{nm}")
                    nc.vector.scalar_tensor_tensor(
                        out=vv[:], in0=tt[:], scalar=float(lim + 3), in1=c1[:],
                        op0=OP.is_le, op1=OP.mult,
                    )
                    return vv

                rv = valid(y0, wl, "r")
                cv = valid(x0, wl, "c")
                mm = pp.tile([P, NG * PS * PS], bf16, tag=f"m_{l}")
                nc.vector.tensor_tensor(
                    out=mm[:].rearrange("p (g a b) -> p g a b", a=PS, b=PS),
                    in0=rv[:].rearrange("p (g k) -> p g k", k=PS)[:, :, :, None]
                        .to_broadcast([P, NG, PS, PS]),
                    in1=cv[:].rearrange("p (g k) -> p g k", k=PS)[:, :, None, :]
                        .to_broadcast([P, NG, PS, PS]),
                    op=OP.mult,
                )
                m_l.append(mm)

            # ---- main pipeline over the 8 groups ----
            # k-swept matmul blocks: within a block, all chunks run k=0 then all
            # k=1 so the stationary f1 tile is loaded twice per block instead of
            # per-matmul (LDWEIGHTS sits on the PE pipeline between matmuls).
            def mm_block(g, chunks):
                # chunks: list of (psum_ap, rhs_tile, rhs_off, width)
                for k in range(2):
                    for pt, rhs, c0, wid in chunks:
                        for s0 in range(0, wid, 512):
                            sw = min(512, wid - s0)
                            nc.tensor.matmul(
                                out=pt[:, s0:s0 + sw],
                                lhsT=f1t[k][:, g * P:(g + 1) * P],
                                rhs=rhs[k][:, c0 + s0:c0 + s0 + sw],
                                start=(k == 0), stop=(k == 1),
                            )

            def do_B(g, corrB):
                ptB1 = psA.tile([P, 1024], f32, tag="mmA", name=f"mmB1_{g}")
                ptB2 = psB.tile([P, 320], f32, tag="mmB", name=f"mmB2_{g}")
                mm_block(g, [(ptB1, f2B, 0, 1024), (ptB2, f2B, 1024, 320)])
                nc.scalar.copy(out=corrB[:, PADB0:PADB0 + 1024], in_=ptB1[:])
                nc.vector.tensor_copy(
                    out=corrB[:, PADB0 + 1024:PADB0 + 1344], in_=ptB2[:])
                nc.sync.dma_start(
                    slabB[g].ap().rearrange("(p f) -> p f", f=ROWB), corrB[:])

            def do_A(g, corrA):
                # two blocks of two 1024-chunks (psA bufs=3 caps live tiles)
                for blk in range(2):
                    pts = [psA.tile([P, 1024], f32, tag="mmA", name=f"mmA_{g}_{blk}_{ci}")
                           for ci in range(2)]
                    mm_block(g, [(pts[ci], f2A, (blk * 2 + ci) * 1024, 1024)
                                 for ci in range(2)])
                    for ci in range(2):
                        c0 = PADA0 + (blk * 2 + ci) * 1024
                        if blk * 2 + ci < 3:
                            nc.scalar.copy(out=corrA[:, c0:c0 + 1024], in_=pts[ci][:])
                        else:
                            nc.vector.tensor_copy(out=corrA[:, c0:c0 + 1024], in_=pts[ci][:])
                nc.sync.dma_start(
                    slabA[g].ap().rearrange("(p f) -> p f", f=ROWA), corrA[:])

            for g in range(NG):
                corrB = corrBb[g % 2]
                corrA = corrAb[g % 2]
                if g < NG - 1:
                    # levels 1-3 first: small matmuls, slabB write, gathers
                    # stream while the level-0 chunks run on the PE
                    do_B(g, corrB)
                    do_A(g, corrA)
                else:
                    # last group: level 0 first so the tail chain is the short
                    # slabB write -> small gathers -> combine
                    do_A(g, corrA)
                    do_B(g, corrB)

                # band gathers (levels 1-3 from slabB first, then level 0; the
                # last group gathers level 0 first to shorten the tail) + pm
                pm4 = pg.tile([P, NLVL * PS * PS], bf16, tag="pm4", name=f"pm4_{g}")
                feats = pg.tile([P, FEAT], bf16, tag="feats", name=f"feats_{g}")

                def gather_pm(l):
                    bl = B_L[l]
                    wl = W_L[l]
                    src = slabA[g] if l == 0 else slabB[g]
                    band = pg.tile([P, bl], bf16, tag=f"band_{l}", name=f"band_{l}_{g}")
                    nc.gpsimd.indirect_dma_start(
                        out=band[:],
                        out_offset=None,
                        in_=src.ap()[:, None],
                        in_offset=bass.IndirectOffsetOnAxis(ap=idx_l[l][:, g:g + 1], axis=0),
                        element_offset=0,
                    )
                    nc.vector.tensor_tensor(
                        out=_ap_view(pm4[:], l * PS * PS, [[PS, PS], [1, PS]]),
                        in0=_ap_view(band[:], 0, [[wl, PS], [1, PS]]),
                        in1=_ap_view(m_l[l][:], g * PS * PS, [[PS, PS], [1, PS]]),
                        op=OP.mult,
                    )

                def taps(l0, nl):
                    # 4-tap bilinear combine over levels [l0, l0+nl), all bf16
                    ov = _ap_view(feats[:], l0 * S * S, [[S * S, nl], [S, S], [1, S]])
                    for t, (a, b) in enumerate(((0, 0), (0, 1), (1, 0), (1, 1))):
                        # feature index = i*9 + j with i = x-offset (outer), j =
                        # y-offset (inner); patch [y=j+a, x=i+b] at (j+a)*10+(i+b)
                        pv = _ap_view(pm4[:], l0 * PS * PS + a * PS + b,
                                      [[PS * PS, nl], [1, S], [PS, S]])
                        wb = _ap_view(w4[t][:], g * NLVL + l0,
                                      [[1, nl], [0, S], [0, S]])
                        if t == 0:
                            nc.vector.tensor_tensor(out=ov, in0=pv, in1=wb, op=OP.mult)
                        else:
                            tmp = pg.tile([P, FEAT], bf16, tag="cmb_tmp")
                            tv = _ap_view(tmp[:], l0 * S * S, [[S * S, nl], [S, S], [1, S]])
                            nc.vector.tensor_tensor(out=tv, in0=pv, in1=wb, op=OP.mult)
                            nc.vector.tensor_tensor(out=ov, in0=ov, in1=tv, op=OP.add)

                if g < NG - 1:
                    for l in (1, 2, 3, 0):
                        gather_pm(l)
                    taps(0, NLVL)
                else:
                    # last group: level-0 combine overlaps the B matmuls, and the
                    # tail chain is only the short slabB write -> B gathers -> taps
                    gather_pm(0)
                    taps(0, 1)
                    for l in (1, 2, 3):
                        gather_pm(l)
                    taps(1, NLVL - 1)

                nc.scalar.dma_start(out_d.ap()[g * P:(g + 1) * P, :], feats[:])

    nc.compile()
    return nc


_NC = None


def _get_nc():
    global _NC
    if _NC is None:
        _NC = build_bass()
    return _NC


def _pool_f2(f2b):
    """f2b: [C, 64, 64] f32 -> [C, 5440] level-concatenated pooled SUMS."""
    lvls = [f2b.reshape(C, HW)]
    cur = f2b
    for _ in range(1, NLVL):
        c, h, w = cur.shape
        cur = cur.reshape(c, h // 2, 2, w // 2, 2).sum(axis=(2, 4))
        lvls.append(cur.reshape(c, -1))
    return np.concatenate(lvls, axis=1)


def make_in_maps(fmap1, fmap2, centroids_coords):
    bf = ml_dtypes.bfloat16
    f2a = [np.ascontiguousarray(_pool_f2(np.asarray(fmap2[bi], dtype=np.float32))).astype(bf)
           for bi in range(2)]
    in_maps = []
    for core in range(8):
        bi, chunk = divmod(core, 4)
        m0 = chunk * NPIX
        f1 = np.ascontiguousarray(
            fmap1[bi].reshape(C, HW)[:, m0:m0 + NPIX]).astype(bf)
        cc = centroids_coords[bi].reshape(2, HW)[:, m0:m0 + NPIX]
        ccx = np.ascontiguousarray(cc[0].reshape(NG, P).T, dtype=np.float32)  # [p, g]
        ccy = np.ascontiguousarray(cc[1].reshape(NG, P).T, dtype=np.float32)
        in_maps.append({"f1": f1, "f2a": f2a[bi], "ccx": ccx, "ccy": ccy})
    return in_maps


def assemble(outs):
    """outs: list of 8 arrays [1024, 324] bf16 -> [2, 324, 64, 64] f32"""
    full = np.empty((2, FEAT, 64, 64), dtype=np.float32)
    for bi in range(2):
        feats = np.concatenate(
            [np.asarray(outs[bi * 4 + c], dtype=np.float32) for c in range(4)], axis=0)
        full[bi] = feats.reshape(64, 64, FEAT).transpose(2, 0, 1)
    return full


def kernel(fmap1, fmap2, centroids_coords, trace=False):
    nc = _get_nc()
    in_maps = make_in_maps(fmap1, fmap2, centroids_coords)
    try:
        res = run_bass_kernel_spmd(nc, in_maps, core_ids=list(range(8)), trace=trace)
    except ModuleNotFoundError:
        res = run_bass_kernel_spmd(nc, in_maps, core_ids=list(range(8)), trace=False)
    out = assemble([r["out"] for r in res.results])
    if trace:
        kernel.last_result = res
    return out
